# revision 29
# baseline (speedup 1.0000x reference)
"""Trainium2 Bass kernel for nn_Attention_14663018349197.

Reference computation (per token t):
    qh = relu(x_q @ Wq);  kh = relu(x_k @ Wk);  vh = relu(x_v @ Wv)
    q2,k2,v2 = reshape to [NV=8 heads, DV=64]
    scores[n,m] = <q2[n], k2[m]> / 8        (attention ACROSS heads)
    p = softmax(scores, axis=m)
    ctx[n] = sum_m p[n,m] v2[m]
    out = relu((ctx + x_q) @ Wfc)

Strategy: pure data-parallel over batch across 8 cores (128 samples each,
11008 tokens/core).  All matmuls in bf16 on TensorE (token-major output via
transposed-x stationaries).  The per-token 8x8x64 cross-head attention runs
on VectorE as two broadcast-AP multiplies (bf16 2x mode) plus halving-tree
adds; softmax on ScalarE (exp with fused 1/8 scale).  Wv's columns are
permuted host-side ( (m,d) -> (d,m) ) so the p*v product keeps innermost
stride-1 operands, which is what keeps VectorE in its 2x perf mode.
"""

import numpy as np
import ml_dtypes

import concourse.bass as bass
import concourse.mybir as mybir
from concourse.bass import ts
from concourse.tile import TileContext
from concourse.bass_utils import run_bass_kernel_spmd
from concourse.masks import make_identity

B, L, D, DV, NV = 1024, 86, 512, 64, 8
NCORES = 8
BS = B // NCORES            # 128 samples per core
T = BS * L                  # 11008 tokens per core
P = 128                     # partitions / tokens per tile
NT = T // P                 # 86 tiles per core
CK = D // P                 # 4 contraction chunks of 128

BF = mybir.dt.bfloat16
F32 = mybir.dt.float32
MUL = mybir.AluOpType.mult
ADD = mybir.AluOpType.add


def split_excess_waits(nc, max_waits=1):
    """This container's walrus build only supports one sync-wait command per
    instruction; hoist extras onto NOPs inserted just before, same engine."""
    n_split = 0
    for f in nc.m.functions:
        for blk in f.blocks:
            new_list = []
            for ins in list(blk.instructions):
                si = getattr(ins, "sync_info", None)
                if si is not None and si.on_wait and len(si.on_wait) > max_waits:
                    waits = list(si.on_wait)
                    keep = waits[-max_waits:]
                    excess = waits[:-max_waits]
                    for i in range(0, len(excess), max_waits):
                        chunk = excess[i : i + max_waits]
                        nop = mybir.InstNoOp(
                            name=nc.get_next_instruction_name(), ins=[], outs=[]
                        )
                        nop.engine = ins.engine
                        nop.sync_info = mybir.SyncInfo(on_wait=chunk, on_update=[])
                        new_list.append(nop)
                        n_split += 1
                    si.on_wait = keep
                new_list.append(ins)
            blk.instructions[:] = new_list
    return n_split


def build_nc(nt=NT, with_bias=False, split_waits=True, repeat=1):
    nc = bass.Bass(trn_type="TRN2")
    t_total = nt * P

    q_d = nc.declare_dram_parameter("query", [t_total, D], F32, isOutput=False)
    k_d = nc.declare_dram_parameter("key", [t_total, D], F32, isOutput=False)
    v_d = nc.declare_dram_parameter("value", [t_total, D], F32, isOutput=False)
    wq_d = nc.declare_dram_parameter("Wq", [D, D], BF, isOutput=False)
    wk_d = nc.declare_dram_parameter("Wk", [D, D], BF, isOutput=False)
    wv_d = nc.declare_dram_parameter("Wvp", [D, D], BF, isOutput=False)
    wfc_d = nc.declare_dram_parameter("Wfc", [D, D], BF, isOutput=False)
    if with_bias:
        bq_d = nc.declare_dram_parameter("bq", [1, D], BF, isOutput=False)
        bk_d = nc.declare_dram_parameter("bk", [1, D], BF, isOutput=False)
        bv_d = nc.declare_dram_parameter("bvp", [1, D], BF, isOutput=False)
        bfc_d = nc.declare_dram_parameter("bfc", [1, D], BF, isOutput=False)
    out_d = nc.declare_dram_parameter("out", [t_total, D], F32, isOutput=True)

    Exp = mybir.ActivationFunctionType.Exp
    Relu = mybir.ActivationFunctionType.Relu

    with TileContext(nc) as tc:
        with (
            tc.tile_pool(name="const", bufs=1) as cpool,
            tc.tile_pool(name="sbuf", bufs=4) as pool,
            tc.tile_pool(name="psum_qkv", bufs=2, space="PSUM") as ps_qkv,
            tc.tile_pool(name="psum_fc", bufs=2, space="PSUM") as ps_fc,
            tc.tile_pool(name="psum_xtp", bufs=2, space="PSUM") as ps_xtp,
            tc.tile_pool(name="psum_ctp", bufs=2, space="PSUM") as ps_ctp,
        ):
            # --- constants: weights, identity ---
            ident = cpool.tile([P, P], BF)
            make_identity(nc, ident)

            w_sb = {}
            for name, wd in (("wq", wq_d), ("wk", wk_d), ("wv", wv_d), ("wfc", wfc_d)):
                w = cpool.tile([P, CK, D], BF, tag=f"w_{name}")
                nc.sync.dma_start(w, wd.rearrange("(ck p) j -> p ck j", p=P))
                w_sb[name] = w
            if with_bias:
                ones_sb = cpool.tile([1, P], BF)
                nc.vector.memset(ones_sb, 1.0)
                b_sb = {}
                for name, bd in (
                    ("wq", bq_d), ("wk", bk_d), ("wv", bv_d), ("wfc", bfc_d)
                ):
                    bt = cpool.tile([1, D], BF, tag=f"b_{name}")
                    nc.sync.dma_start(bt, bd)
                    b_sb[name] = bt

            def proj_matmuls(ps, xT, wname, start=True, stop=True):
                n_mm = CK + (1 if with_bias else 0)
                i_mm = 0
                for ck in range(CK):
                    nc.tensor.matmul(
                        ps,
                        lhsT=xT[:, ck],
                        rhs=w_sb[wname][:, ck],
                        start=(start and i_mm == 0),
                        stop=(stop and i_mm == n_mm - 1),
                    )
                    i_mm += 1
                if with_bias:
                    nc.tensor.matmul(
                        ps,
                        lhsT=ones_sb,
                        rhs=b_sb[wname],
                        start=False,
                        stop=stop,
                    )

            for it in range(nt * repeat):
                it = it % nt
                tok = ts(it, P)

                # --- load (f32 -> bf16 cast in DMA), token-major ---
                xq = pool.tile([P, D], BF, tag="xq")
                xk = pool.tile([P, D], BF, tag="xk")
                xv = pool.tile([P, D], BF, tag="xv")
                nc.gpsimd.dma_start(out=xq, in_=q_d[tok])
                nc.gpsimd.dma_start(out=xk, in_=k_d[tok])
                nc.gpsimd.dma_start(out=xv, in_=v_d[tok])

                # --- transpose x to feature-major chunks (PE) ---
                xTs = []
                for j, x in enumerate((xq, xk, xv)):
                    xT_ps = ps_xtp.tile([P, CK, P], BF, tag="xtp")
                    for ck in range(CK):
                        nc.tensor.transpose(xT_ps[:, ck], x[:, ts(ck, P)], ident)
                    xTj = pool.tile([P, CK, P], BF, tag=f"xT{j}")
                    nc.scalar.copy(out=xTj, in_=xT_ps)
                    xTs.append(xTj)
                xqT, xkT, xvT = xTs

                # --- projections (PE) + relu-cast (ACT) ---
                qh_ps = ps_qkv.tile([P, D], F32, tag="qkv")
                proj_matmuls(qh_ps, xqT, "wq")
                qh = pool.tile([P, D], BF, tag="qh")
                nc.scalar.activation(qh, qh_ps, Relu)

                kh_ps = ps_qkv.tile([P, D], F32, tag="qkv")
                proj_matmuls(kh_ps, xkT, "wk")
                kh = pool.tile([P, D], BF, tag="kh")
                nc.scalar.activation(kh, kh_ps, Relu)

                vh_ps = ps_qkv.tile([P, D], F32, tag="qkv")
                proj_matmuls(vh_ps, xvT, "wv")
                vh = pool.tile([P, D], BF, tag="vh")
                nc.scalar.activation(vh, vh_ps, Relu)

                # --- scores: E = q2 (x) k2, tree-reduce over d (DVE) ---
                E = pool.tile([P, NV * NV * DV], BF, tag="E")
                E4 = E.rearrange("p (n m d) -> p n m d", n=NV, m=NV, d=DV)
                qh4 = qh.rearrange("p (n d) -> p n d", n=NV)[:, :, None, :].to_broadcast(
                    (P, NV, NV, DV)
                )
                kh4 = kh.rearrange("p (m d) -> p m d", m=NV)[:, None, :, :].to_broadcast(
                    (P, NV, NV, DV)
                )
                nc.vector.tensor_tensor(E4, qh4, kh4, MUL)

                # halving tree over d: 64->1
                cur = E.rearrange("p (nm d) -> p nm d", nm=NV * NV)
                w = DV
                while w > 2:
                    nxt = pool.tile([P, NV * NV, w // 2], BF, tag=f"st{w}")
                    nc.vector.tensor_tensor(
                        nxt, cur[:, :, : w // 2], cur[:, :, w // 2 :], ADD
                    )
                    cur = nxt
                    w //= 2
                scores = pool.tile([P, NV * NV], F32, tag="scores")
                nc.vector.tensor_tensor(
                    scores.rearrange("p (nm o) -> p nm o", o=1),
                    cur[:, :, 0:1],
                    cur[:, :, 1:2],
                    ADD,
                )

                # --- softmax over m (ACT exp + DVE) ---
                e = pool.tile([P, NV * NV], BF, tag="e")
                nc.scalar.activation(e, scores, Exp, scale=1.0 / np.sqrt(DV))
                den = pool.tile([P, NV], F32, tag="den")
                nc.vector.tensor_reduce(
                    den,
                    e.rearrange("p (n m) -> p n m", n=NV),
                    axis=mybir.AxisListType.X,
                    op=ADD,
                )
                rec = pool.tile([P, NV], F32, tag="rec")
                nc.vector.reciprocal(rec, den)
                p_at = pool.tile([P, NV * NV], BF, tag="p_at")
                nc.vector.tensor_tensor(
                    p_at.rearrange("p (n m) -> p n m", n=NV),
                    e.rearrange("p (n m) -> p n m", n=NV),
                    rec[:, :, None].to_broadcast((P, NV, NV)),
                    MUL,
                )

                # --- ctx: F = p (x) v2', tree-reduce over m (DVE) ---
                # vh layout is (d, m) thanks to host-side Wv column permutation.
                F = pool.tile([P, NV * DV * NV], BF, tag="F")
                F4 = F.rearrange("p (n d m) -> p n d m", n=NV, d=DV, m=NV)
                vh4 = vh.rearrange("p (d m) -> p d m", d=DV)[:, None, :, :].to_broadcast(
                    (P, NV, DV, NV)
                )
                p4 = p_at.rearrange("p (n m) -> p n m", n=NV)[:, :, None, :].to_broadcast(
                    (P, NV, DV, NV)
                )
                nc.vector.tensor_tensor(F4, vh4, p4, MUL)

                curf = F.rearrange("p (nd m) -> p nd m", nd=NV * DV)
                w = NV
                while w > 2:
                    nxt = pool.tile([P, NV * DV, w // 2], BF, tag=f"ct{w}")
                    nc.vector.tensor_tensor(
                        nxt, curf[:, :, : w // 2], curf[:, :, w // 2 :], ADD
                    )
                    curf = nxt
                    w //= 2

                # ctx = A + B (A=curf[...,0], B=curf[...,1]); instead of adding
                # on DVE, transpose both (strided stationaries) and let the FC
                # accumulation absorb the add: (A+B+x_q) @ Wfc.
                ctxT_ps = ps_ctp.tile([P, 2 * CK, P], BF, tag="ctp")
                for half in range(2):
                    for ck in range(CK):
                        nc.tensor.transpose(
                            ctxT_ps[:, half * CK + ck],
                            curf[:, ts(ck, P), half],
                            ident,
                        )
                ctxT = pool.tile([P, 2 * CK, P], BF, tag="ctxT")
                nc.scalar.copy(out=ctxT, in_=ctxT_ps)

                fc_ps = ps_fc.tile([P, D], F32, tag="fc")
                proj_matmuls(fc_ps, ctxT[:, 0:CK], "wfc", start=True, stop=False)
                proj_matmuls(fc_ps, ctxT[:, CK : 2 * CK], "wfc", start=False, stop=False)
                # residual: (ctx + x_q) @ Wfc == ctx@Wfc + x_q@Wfc
                for ck in range(CK):
                    nc.tensor.matmul(
                        fc_ps,
                        lhsT=xqT[:, ck],
                        rhs=w_sb["wfc"][:, ck],
                        start=False,
                        stop=(ck == CK - 1),
                    )

                out_sb = pool.tile([P, D], F32, tag="out")
                nc.scalar.activation(out_sb, fc_ps, Relu)
                nc.sync.dma_start(out=out_d[tok], in_=out_sb)

    if split_waits:
        split_excess_waits(nc)
    return nc


def _wv_perm():
    # j' = d*NV + m  <-  j = m*DV + d
    jp = np.arange(D)
    d, m = jp // NV, jp % NV
    return m * DV + d


def make_in_maps(inputs, with_bias):
    bf = ml_dtypes.bfloat16
    perm = _wv_perm()
    wq = inputs["Wq"].astype(bf)
    wk = inputs["Wk"].astype(bf)
    wvp = inputs["Wv"][:, perm].astype(bf)
    wfc = inputs["Wfc"].astype(bf)
    common = {"Wq": wq, "Wk": wk, "Wvp": wvp, "Wfc": wfc}
    if with_bias:
        common["bq"] = inputs["bq"].reshape(1, D).astype(bf)
        common["bk"] = inputs["bk"].reshape(1, D).astype(bf)
        common["bvp"] = inputs["bv"][perm].reshape(1, D).astype(bf)
        common["bfc"] = inputs["bfc"].reshape(1, D).astype(bf)
    in_maps = []
    for c in range(NCORES):
        sl = slice(c * BS, (c + 1) * BS)
        in_maps.append(
            {
                "query": np.ascontiguousarray(
                    inputs["query"][sl].reshape(T, D), dtype=np.float32
                ),
                "key": np.ascontiguousarray(
                    inputs["key"][sl].reshape(T, D), dtype=np.float32
                ),
                "value": np.ascontiguousarray(
                    inputs["value"][sl].reshape(T, D), dtype=np.float32
                ),
                **common,
            }
        )
    return in_maps


def kernel(**inputs):
    inputs = {k: np.asarray(v) for k, v in inputs.items()}
    with_bias = any(
        np.abs(inputs[b]).max() > 0 for b in ("bq", "bk", "bv", "bfc")
    )
    nc = build_nc(nt=NT, with_bias=with_bias)
    in_maps = make_in_maps(inputs, with_bias)
    res = run_bass_kernel_spmd(nc, in_maps, core_ids=list(range(NCORES)))
    out = np.concatenate(
        [res.results[c]["out"].reshape(BS, L, D) for c in range(NCORES)], axis=0
    )
    return out.astype(np.float32)


if __name__ == "__main__":
    nc = build_nc(nt=2)
    print("built ok:", sum(len(b.instructions) for f in nc.m.functions for b in f.blocks), "instructions")
